# revision 1
# baseline (speedup 1.0000x reference)
"""Causal multi-head attention (B=4, S=2048, E=1024, H=16) on 8 trn2 NeuronCores.

Sharding: core c = (batch b = c//2, head-group g = c%2). Each core computes
attention for one batch element and 8 of the 16 heads, plus the partial
out-projection restricted to its heads' columns. Host sums the two partial
outputs per batch element and adds the out-projection bias.

Everything on-device flows in "transposed" space so no on-device transposes
are needed:
  qT, kT  [ch, s]   from  W_chunk @ x.T        (lhsT = W.T tiles, rhs = x.T)
  v       [s, ch]   from  x @ Wv.T             (lhsT = x.T tiles, rhs = Wv.T)
  scoresT [k, q]    from  lhsT = kT, rhs = qT  (per head, K = 64)
  ctxT    [d, q]    from  lhsT = v (+ones col), rhs = exp(scoresT)
  outP    [s, o]    from  lhsT = ctxT, rhs = Wo.T
Softmax is computed without max subtraction (scores are bounded ~|2|), the
normalizer comes from a ones-column appended to v, and causal masking is a
multiplicative 0/1 triangular mask on the exp'd diagonal blocks.
"""

import sys

sys.path.insert(0, "/opt/trn_rl_repo")

import numpy as np

import concourse.bass as bass  # noqa: F401  (registers engine classes)
import concourse.mybir as mybir
import concourse.tile as tile
from concourse import bacc
from concourse.bass_utils import run_bass_kernel_spmd

F32 = mybir.dt.float32
F32R = mybir.dt.float32r
AF = mybir.ActivationFunctionType

B, S, E = 4, 2048, 1024
H, HD = 16, 64
GH = 8                 # heads handled per core
GC = GH * HD           # 512 channels per head-group
P = 128
NCORES = 8
NJ_ALL = S // P        # 16 k-blocks of 128
QB = S // 512          # 4 q-windows of 512

_program = {}


def _r(ap):
    return ap.bitcast(F32R)


def _emit(tc, nc, xT, wqkT, wvT, woT, bqk, bv, out, bench_iters=0, has_bias=True):
    ctxmgr = []

    def pool(**kw):
        p = tc.tile_pool(**kw)
        ctxmgr.append(p)
        return p.__enter__()

    const = pool(name="const", bufs=1)
    kvp = pool(name="kv", bufs=1)
    xp = pool(name="xs", bufs=2)
    qp = pool(name="qt", bufs=2)
    cxp = pool(name="ctx", bufs=2)
    ep = pool(name="expt", bufs=3)
    osb = pool(name="osb", bufs=2)
    bp = pool(name="bcast", bufs=2)
    ps_s = pool(name="ps_s", bufs=2, space="PSUM")
    ps_m = pool(name="ps_m", bufs=4, space="PSUM")

    # ---- constants ----
    # DMA order matters at startup: the first qkT matmuls need wqk + the
    # first x strip; wo is only needed ~100us in (first out-projection),
    # so it is emitted last.
    bqk_sb = const.tile([P, 8], F32)
    nc.sync.dma_start(bqk_sb[:], bqk.rearrange("c p -> p c"))
    bv_sb = const.tile([P, 4], F32)
    nc.sync.dma_start(bv_sb[:], bv.rearrange("c p -> p c"))
    # Per-chunk DMAs so the first qkT matmul (which only reads chunk e=0)
    # can start as soon as its chunk lands, not after the whole 4MB.
    wqk_sb = const.tile([P, 8, 2 * GC], F32R)      # [p, e, ch]: W row e*128+p
    wqk_r = wqkT.rearrange("(eo p) c -> p eo c", p=P)
    for e in range(8):
        eng = nc.sync if e % 2 == 0 else nc.gpsimd
        eng.dma_start(wqk_sb[:, e, :], wqk_r[:, e, :])
    wv_sb = const.tile([P, 8, GC], F32R)
    wv_r = wvT.rearrange("(eo p) c -> p eo c", p=P)
    for e in range(8):
        eng = nc.gpsimd if e % 2 == 0 else nc.sync
        eng.dma_start(wv_sb[:, e, :], wv_r[:, e, :])
    wo_sb = const.tile([P, 4, E], F32R)
    wo_r = woT.rearrange("(co p) o -> p co o", p=P)
    for co in range(4):
        nc.gpsimd.dma_start(wo_sb[:, co, :], wo_r[:, co, :])

    # Shifted causal mask bank: mbig[p, g] = 1 if g - p - 384 >= 0 else 0.
    # For a diagonal k-block with offset t (t = j - 4*qb), the mask over the
    # first (t+1)*128 q-columns is mbig[:, 384-128t : 512] — it zeroes the
    # fully-masked left columns and the strict lower triangle of the band.
    # Built in f32 (memset/affine_select don't take f32r), then cast.
    tmpp = tc.tile_pool(name="tmpf", bufs=1)
    tmp = tmpp.__enter__()
    mbig_f = tmp.tile([P, 896], F32)
    nc.gpsimd.memset(mbig_f[:], 1.0)
    nc.gpsimd.affine_select(
        out=mbig_f[:],
        in_=mbig_f[:],
        compare_op=mybir.AluOpType.is_ge,
        fill=0.0,
        base=-384,
        pattern=[[1, 896]],      # + g
        channel_multiplier=-1,   # - p   => keep where g - p - 384 >= 0
    )
    mbig = const.tile([P, 896], F32R)
    nc.vector.tensor_copy(mbig[:], mbig_f[:])

    # ---- persistent K^T and V ----
    kT_sb = kvp.tile([P, 4, S], F32R)            # [p, c, s]; ch = c*128+p
    v_sb = kvp.tile([P, NJ_ALL, GH, HD + 1], F32R)  # [s%128, j, h, d(+ones)]
    ones_f = tmp.tile([P, NJ_ALL * GH], F32)
    nc.vector.memset(ones_f[:], 1.0)
    nc.vector.tensor_copy(
        v_sb[:, :, :, HD],
        ones_f[:].rearrange("p (j h) -> p j h", j=NJ_ALL),
    )
    tmpp.__exit__(None, None, None)

    xTr = xT.rearrange("(eo p) s -> p eo s", p=P)

    if not has_bias:
        bqk_sb = bv_sb = None
    import contextlib
    if bench_iters:
        # large body (>256 insts/engine): arm branch prefetch so the
        # back-edge doesn't stall on an IRAM refetch every iteration
        loop_cm = tc.For_i(0, bench_iters, 1,
                           hint_engines=(mybir.EngineType.PE,
                                         mybir.EngineType.DVE,
                                         mybir.EngineType.Activation,
                                         mybir.EngineType.Pool,
                                         mybir.EngineType.SP))
    else:
        loop_cm = contextlib.nullcontext()
    with loop_cm:
        _emit_body(tc, nc, xTr, out, wqk_sb, wv_sb, wo_sb, bqk_sb, bv_sb,
                   mbig, kT_sb, v_sb, qp, xp, cxp, ep, osb, bp, ps_s, ps_m)

    for p in reversed(ctxmgr):
        p.__exit__(None, None, None)


def _emit_body(tc, nc, xTr, out, wqk_sb, wv_sb, wo_sb, bqk_sb, bv_sb,
               mbig, kT_sb, v_sb, qp, xp, cxp, ep, osb, bp, ps_s, ps_m):
    """Software-pipelined emission: attention(qb) is the ACT-paced backbone;
    PE-only work — qkv(qb+1) chains and outproj(qb-1) chains — is spliced
    between individual j-iterations so the in-order PE stream always has
    independent matmuls to chew on while it waits for exp results."""
    def new_qT(qb):
        qT = qp.tile([P, 4, 512], F32R, tag="qT", name=f"qT{qb % 2}")
        return qT

    def qkv_chains(qb, qT):
        # round-robin the two strips so strip 1's x DMA is issued while
        # strip 0's chains are still running
        yield from _roundrobin([
            _qkv_strip_chains(tc, nc, xTr, wqk_sb, wv_sb, bqk_sb,
                              kT_sb, v_sb, xp, ps_m, qb, ss, qT)
            for ss in range(2)])

    def run_chains(gen):
        for chain in gen:
            chain()

    qT_cur = new_qT(0)
    run_chains(qkv_chains(0, qT_cur))
    ctx_prev = None
    for qb in range(QB):
        qT_next = new_qT(qb + 1) if qb + 1 < QB else None
        fillers = []
        if qT_next is not None:
            fillers.append(qkv_chains(qb + 1, qT_next))
        if ctx_prev is not None:
            fillers.append(_outproj_chains(tc, nc, out, wo_sb, ctx_prev,
                                           osb, ps_m, qb - 1))
        ctx_prev = _attn(tc, nc, mbig, kT_sb, v_sb, bv_sb, qT_cur,
                         cxp, ep, bp, ps_s, ps_m, qb,
                         fillers=_roundrobin(fillers))
        qT_cur = qT_next
    run_chains(_outproj_chains(tc, nc, out, wo_sb, ctx_prev, osb, ps_m,
                               QB - 1))


def _roundrobin(gens):
    gens = list(gens)
    while gens:
        g = gens.pop(0)
        try:
            yield next(g)
            gens.append(g)
        except StopIteration:
            pass


def _qkv_strip_chains(tc, nc, xTr, wqk_sb, wv_sb, bqk_sb, kT_sb, v_sb,
                      xp, ps_m, qb, ss, qT):
    """Yield one callable per accumulation chain (8 matmuls + a drain op)."""
    q0 = qb * 512
    s0 = q0 + ss * 256
    state = {}

    def load_x():
        xs = xp.tile([P, 8, 256], F32R)
        nc.sync.dma_start(xs[:], xTr[:, :, s0:s0 + 256])
        state["xs"] = xs

    yield load_x

    def qk_chain(cb):
        xs = state["xs"]
        pq = ps_m.tile([P, 512], F32, tag="m")
        for e in range(8):
            nc.tensor.matmul(
                pq[:, 0:256],
                (wqk_sb[:, e, cb * P:(cb + 1) * P]),
                (xs[:, e, :]),
                start=(e == 0), stop=(e == 7),
            )
        if cb < 4:
            dest = qT[:, cb, ss * 256:(ss + 1) * 256]
        else:
            dest = kT_sb[:, cb - 4, s0:s0 + 256]
        if bqk_sb is not None:
            nc.vector.tensor_scalar_add(dest, pq[:, 0:256],
                                        bqk_sb[:, cb:cb + 1])
        else:
            nc.vector.tensor_copy(dest, pq[:, 0:256])

    def v_chain(sv):
        xs = state["xs"]
        j = s0 // P + sv
        pv = ps_m.tile([P, 512], F32, tag="m")
        for e in range(8):
            nc.tensor.matmul(
                pv[:],
                (xs[:, e, sv * P:(sv + 1) * P]),
                (wv_sb[:, e, :]),
                start=(e == 0), stop=(e == 7),
            )
        nc.vector.tensor_copy(
            v_sb[:, j, :, 0:HD],
            pv[:].rearrange("p (h d) -> p h d", h=GH),
        )

    # k and v chains first: the next qb's attention needs kT/v before qT
    for cb in (4, 5, 6, 7):
        yield (lambda cb=cb: qk_chain(cb))
    for sv in range(2):
        yield (lambda sv=sv: v_chain(sv))
    for cb in (0, 1, 2, 3):
        yield (lambda cb=cb: qk_chain(cb))


def _emit_pv(nc, pv2, v_sb, c, j, w0, ex, nj):
    for hp in range(2):
        nc.tensor.matmul(
            pv2[hp][0:HD + 1, w0:512],
            (v_sb[:, j, 2 * c + hp, :]),
            (ex[:, hp, w0:512]),
            start=(j == 0), stop=(j == nj - 1),
        )


def _outproj_chains(tc, nc, out, wo_sb, ctxT, osb, ps_m, qb):
    q0 = qb * 512
    for sb_i in range(4):
        for ob in range(2):
            def chain(sb_i=sb_i, ob=ob):
                po = ps_m.tile([P, 512], F32, tag="m")
                for cc in range(4):
                    nc.tensor.matmul(
                        po[:],
                        (ctxT[:, cc, sb_i * P:(sb_i + 1) * P]),
                        (wo_sb[:, cc, ob * 512:(ob + 1) * 512]),
                        start=(cc == 0), stop=(cc == 3),
                    )
                ot = osb.tile([P, 512], F32)
                nc.vector.tensor_copy(ot[:], po[:])
                nc.sync.dma_start(
                    out[q0 + sb_i * P:q0 + (sb_i + 1) * P,
                        ob * 512:(ob + 1) * 512],
                    ot[:],
                )
            yield chain


def _attn(tc, nc, mbig, kT_sb, v_sb, bv_sb, qT, cxp, ep, bp, ps_s, ps_m, qb, fillers=None):
    # ---- attention for this q-window ----
    # Heads 2c (SBUF partitions 0-63) and 2c+1 (64-127) are processed
    # together: their score matmuls land on PE row-groups (0,0)/(64,0)
    # and overlap in the array.
    ctxT = cxp.tile([P, 4, 512], F32R)       # [p, c, q]; ch = c*128+p
    nj = 4 * (qb + 1)
    fillers = iter(fillers) if fillers is not None else iter(())
    done = False
    n_iters = 4 * nj
    acc = 0.0
    per_iter = 24.0 / n_iters   # ~30 filler chains spread over the window

    def emit_fillers(force_all=False):
        nonlocal acc, done
        if done:
            return
        acc += per_iter
        while (acc >= 1.0 or force_all) and not done:
            acc -= 1.0
            try:
                next(fillers)()
            except StopIteration:
                done = True

    if True:
        for c in range(4):
            pv2 = [ps_m.tile([P, 512], F32, tag="m", name=f"pv{hp}")
                   for hp in range(2)]
            pend = None   # software-pipeline: PV trails scores by one j
            for j in range(nj):
                t = j - 4 * qb
                # Diagonal blocks only need q-columns >= t*128 (causality);
                # fp32r matmuls need N >= 256 for full rate, so the window
                # is capped at 256 wide minimum.
                w0 = 0 if t < 0 else min(t * P, 256)
                sp = ps_s.tile([P, 2, 512], F32)
                for hp in range(2):
                    p0 = 64 * hp
                    nc.tensor.matmul(
                        sp[:, hp, w0:512],
                        (kT_sb[p0:p0 + 64, c, j * P:(j + 1) * P]),
                        (qT[p0:p0 + 64, c, w0:512]),
                        start=True, stop=True,
                    )
                ex = ep.tile([P, 2, 512], F32R)
                nc.scalar.activation(ex[:, :, w0:512], sp[:, :, w0:512], AF.Exp)
                if t >= 0:
                    # mask multiply over the diagonal band (plus, for t=3,
                    # the fully-masked in-window left columns); columns right
                    # of the band are untouched (mask would be all-ones)
                    m0 = t * P if t < 3 else 256
                    m1 = (t + 1) * P if t < 3 else 512
                    nc.vector.tensor_mul(
                        ex[:, :, m0:m1],
                        ex[:, :, m0:m1],
                        mbig[:, None, m0 - 128 * t + 384:m1 - 128 * t + 384]
                        .to_broadcast((P, 2, m1 - m0)),
                    )
                if pend is not None:
                    _emit_pv(nc, pv2, v_sb, c, *pend, nj)
                pend = (j, w0, ex)
                emit_fillers()
            if pend is not None:
                _emit_pv(nc, pv2, v_sb, c, *pend, nj)
            # normalize: ctxT = pv[0:64] / pv[64] (+ v bias)
            for hp in range(2):
                p0 = 64 * hp
                pv_ps = pv2[hp]
                bc = bp.tile([64, 512], F32)
                nc.vector.reciprocal(bc[0:1, :], pv_ps[HD:HD + 1, :])
                nc.gpsimd.partition_broadcast(bc[:], bc[0:1, :])
                nc.vector.tensor_mul(ctxT[p0:p0 + 64, c, :], pv_ps[0:HD, :], bc[:])
                if bv_sb is not None:
                    nc.vector.tensor_scalar_add(
                        ctxT[p0:p0 + 64, c, :],
                        ctxT[p0:p0 + 64, c, :],
                        bv_sb[p0:p0 + 64, c:c + 1],
                    )
    emit_fillers(force_all=True)
    return ctxT


def _build_program(bench_iters=0, has_bias=True):
    nc = bacc.Bacc("TRN2", target_bir_lowering=False, debug=False,
                   num_devices=NCORES)
    xT = nc.dram_tensor("xT", [E, S], F32R, kind="ExternalInput").ap()
    wqkT = nc.dram_tensor("wqkT", [E, 2 * GC], F32R, kind="ExternalInput").ap()
    wvT = nc.dram_tensor("wvT", [E, GC], F32R, kind="ExternalInput").ap()
    woT = nc.dram_tensor("woT", [GC, E], F32R, kind="ExternalInput").ap()
    bqk = nc.dram_tensor("bqk", [8, P], F32, kind="ExternalInput").ap()
    bv = nc.dram_tensor("bv", [4, P], F32, kind="ExternalInput").ap()
    out = nc.dram_tensor("o", [S, E], F32, kind="ExternalOutput").ap()
    with tile.TileContext(nc) as tc:
        _emit(tc, nc, xT, wqkT, wvT, woT, bqk, bv, out, bench_iters=bench_iters,
              has_bias=has_bias)
    nc.compile()
    return nc


def _get_program(has_bias=True):
    if has_bias not in _program:
        _program[has_bias] = _build_program(has_bias=has_bias)
    return _program[has_bias]


def _make_in_maps(x, in_proj_w, in_proj_b, out_proj_w):
    scale = np.float32(1.0 / np.sqrt(HD))
    in_maps = []
    for c in range(NCORES):
        b, g = divmod(c, 2)
        lo, hi = g * GC, (g + 1) * GC
        wq = in_proj_w[lo:hi, :]
        wk = in_proj_w[E + lo:E + hi, :]
        wv = in_proj_w[2 * E + lo:2 * E + hi, :]
        wqkT = np.concatenate([wq.T * scale, wk.T], axis=1)
        wvT = np.ascontiguousarray(wv.T)
        woT = np.ascontiguousarray(out_proj_w[:, lo:hi].T)
        bqk = np.concatenate([in_proj_b[lo:hi] * scale,
                              in_proj_b[E + lo:E + hi]]).reshape(8, P)
        bvv = in_proj_b[2 * E + lo:2 * E + hi].reshape(4, P)
        xT = np.ascontiguousarray(x[b].T)
        in_maps.append({
            "xT": np.ascontiguousarray(xT, dtype=np.float32),
            "wqkT": np.ascontiguousarray(wqkT, dtype=np.float32),
            "wvT": wvT.astype(np.float32, copy=False),
            "woT": woT.astype(np.float32, copy=False),
            "bqk": np.ascontiguousarray(bqk, dtype=np.float32),
            "bv": np.ascontiguousarray(bvv, dtype=np.float32),
        })
    return in_maps


def _combine(results, out_proj_b):
    out = np.empty((B, S, E), dtype=np.float32)
    for b in range(B):
        out[b] = results[2 * b]["o"] + results[2 * b + 1]["o"]
    out += np.asarray(out_proj_b, dtype=np.float32)[None, None, :]
    return out


def kernel(x, in_proj_w, in_proj_b, out_proj_w, out_proj_b, _trace=False):
    x = np.asarray(x, dtype=np.float32)
    in_proj_w = np.asarray(in_proj_w, dtype=np.float32)
    in_proj_b = np.asarray(in_proj_b, dtype=np.float32)
    out_proj_w = np.asarray(out_proj_w, dtype=np.float32)
    out_proj_b = np.asarray(out_proj_b, dtype=np.float32)
    assert x.shape == (B, S, E), x.shape

    has_bias = bool(np.any(in_proj_b))
    nc = _get_program(has_bias=has_bias)
    in_maps = _make_in_maps(x, in_proj_w, in_proj_b, out_proj_w)
    res = run_bass_kernel_spmd(nc, in_maps, core_ids=list(range(NCORES)),
                               trace=_trace)
    out = _combine(res.results, out_proj_b)
    if _trace:
        return out, res
    return out



# revision 5
# speedup vs baseline: 1.1295x; 1.1295x over previous
"""Causal multi-head attention (B=4, S=2048, E=1024, H=16) on 8 trn2 NeuronCores.

Sharding: core c = (batch b = c//2, head-group g = c%2). Each core computes
attention for one batch element and 8 of the 16 heads, plus the partial
out-projection restricted to its heads' columns. Host sums the two partial
outputs per batch element and adds the out-projection bias.

Everything on-device flows in "transposed" space so no on-device transposes
are needed:
  qT, kT  [ch, s]   from  W_chunk @ x.T        (lhsT = W.T tiles, rhs = x.T)
  v       [s, ch]   from  x @ Wv.T             (lhsT = x.T tiles, rhs = Wv.T)
  scoresT [k, q]    from  lhsT = kT, rhs = qT  (per head, K = 64)
  ctxT    [d, q]    from  lhsT = v (+ones col), rhs = exp(scoresT)
  outP    [s, o]    from  lhsT = ctxT, rhs = Wo.T
Softmax is computed without max subtraction (scores are bounded ~|2|), the
normalizer comes from a ones-column appended to v, and causal masking is a
multiplicative 0/1 triangular mask on the exp'd diagonal blocks.
"""

import sys

sys.path.insert(0, "/opt/trn_rl_repo")

import numpy as np

import concourse.bass as bass  # noqa: F401  (registers engine classes)
import concourse.mybir as mybir
import concourse.tile as tile
from concourse import bacc
from concourse.bass_utils import run_bass_kernel_spmd

F32 = mybir.dt.float32
F32R = mybir.dt.float32r
AF = mybir.ActivationFunctionType

B, S, E = 4, 2048, 1024
H, HD = 16, 64
GH = 8                 # heads handled per core
GC = GH * HD           # 512 channels per head-group
P = 128
NCORES = 8
NJ_ALL = S // P        # 16 k-blocks of 128
QB = S // 512          # 4 q-windows of 512

_program = {}


def _r(ap):
    return ap.bitcast(F32R)


def _emit(tc, nc, xT, wqkT, wvT, woT, bqk, bv, out, bench_iters=0, has_bias=True):
    ctxmgr = []

    def pool(**kw):
        p = tc.tile_pool(**kw)
        ctxmgr.append(p)
        return p.__enter__()

    const = pool(name="const", bufs=1)
    kvp = pool(name="kv", bufs=1)
    xp = pool(name="xs", bufs=1)
    qp = pool(name="qt", bufs=2)
    cxp = pool(name="ctx", bufs=2)
    ep = pool(name="expt", bufs=3)
    osb = pool(name="osb", bufs=2)
    bp = pool(name="bcast", bufs=2)
    ps_s = pool(name="ps_s", bufs=2, space="PSUM")
    ps_m = pool(name="ps_m", bufs=4, space="PSUM")

    # ---- constants ----
    # DMA order matters at startup: the first qkT matmuls need wqk + the
    # first x strip; wo is only needed ~100us in (first out-projection),
    # so it is emitted last.
    bqk_sb = const.tile([P, 8], F32)
    nc.sync.dma_start(bqk_sb[:], bqk.rearrange("c p -> p c"))
    bv_sb = const.tile([P, 4], F32)
    nc.sync.dma_start(bv_sb[:], bv.rearrange("c p -> p c"))
    # Per-chunk DMAs so the first qkT matmul (which only reads chunk e=0)
    # can start as soon as its chunk lands, not after the whole 4MB.
    wqk_sb = const.tile([P, 8, 2 * GC], F32R)      # [p, e, ch]: W row e*128+p
    wqk_r = wqkT.rearrange("(eo p) c -> p eo c", p=P)
    for e in range(8):
        eng = nc.sync if e % 2 == 0 else nc.gpsimd
        eng.dma_start(wqk_sb[:, e, :], wqk_r[:, e, :])
    wv_sb = const.tile([P, 8, GC], F32R)
    wv_r = wvT.rearrange("(eo p) c -> p eo c", p=P)
    for e in range(8):
        eng = nc.gpsimd if e % 2 == 0 else nc.sync
        eng.dma_start(wv_sb[:, e, :], wv_r[:, e, :])
    wo_sb = const.tile([P, 4, E], F32R)
    wo_r = woT.rearrange("(co p) o -> p co o", p=P)
    for co in range(4):
        nc.gpsimd.dma_start(wo_sb[:, co, :], wo_r[:, co, :])

    # Shifted causal mask bank: mbig[p, g] = 1 if g - p - 384 >= 0 else 0.
    # For a diagonal k-block with offset t (t = j - 4*qb), the mask over the
    # first (t+1)*128 q-columns is mbig[:, 384-128t : 512] — it zeroes the
    # fully-masked left columns and the strict lower triangle of the band.
    # Built in f32 (memset/affine_select don't take f32r), then cast.
    tmpp = tc.tile_pool(name="tmpf", bufs=1)
    tmp = tmpp.__enter__()
    mbig_f = tmp.tile([P, 896], F32)
    nc.gpsimd.memset(mbig_f[:], 1.0)
    nc.gpsimd.affine_select(
        out=mbig_f[:],
        in_=mbig_f[:],
        compare_op=mybir.AluOpType.is_ge,
        fill=0.0,
        base=-384,
        pattern=[[1, 896]],      # + g
        channel_multiplier=-1,   # - p   => keep where g - p - 384 >= 0
    )
    mbig = const.tile([P, 896], F32R)
    nc.vector.tensor_copy(mbig[:], mbig_f[:])

    # ---- persistent K^T and V ----
    kT_sb = kvp.tile([P, 4, S], F32R)            # [p, c, s]; ch = c*128+p
    v_sb = kvp.tile([P, NJ_ALL, GH, HD + 1], F32R)  # [s%128, j, h, d(+ones)]
    ones_f = tmp.tile([P, NJ_ALL * GH], F32)
    nc.vector.memset(ones_f[:], 1.0)
    nc.vector.tensor_copy(
        v_sb[:, :, :, HD],
        ones_f[:].rearrange("p (j h) -> p j h", j=NJ_ALL),
    )
    tmpp.__exit__(None, None, None)

    xTr = xT.rearrange("(eo p) s -> p eo s", p=P)

    if not has_bias:
        bqk_sb = bv_sb = None
    import contextlib
    if bench_iters:
        # large body (>256 insts/engine): arm branch prefetch so the
        # back-edge doesn't stall on an IRAM refetch every iteration
        loop_cm = tc.For_i(0, bench_iters, 1,
                           hint_engines=(mybir.EngineType.PE,
                                         mybir.EngineType.DVE,
                                         mybir.EngineType.Activation,
                                         mybir.EngineType.Pool,
                                         mybir.EngineType.SP))
    else:
        loop_cm = contextlib.nullcontext()
    with loop_cm:
        _emit_body(tc, nc, xTr, out, wqk_sb, wv_sb, wo_sb, bqk_sb, bv_sb,
                   mbig, kT_sb, v_sb, qp, xp, cxp, ep, osb, bp, ps_s, ps_m)

    for p in reversed(ctxmgr):
        p.__exit__(None, None, None)


def _emit_body(tc, nc, xTr, out, wqk_sb, wv_sb, wo_sb, bqk_sb, bv_sb,
               mbig, kT_sb, v_sb, qp, xp, cxp, ep, osb, bp, ps_s, ps_m):
    """Software-pipelined emission: attention(qb) is the ACT-paced backbone;
    PE-only work — qkv(qb+1) chains and outproj(qb-1) chains — is spliced
    between individual j-iterations so the in-order PE stream always has
    independent matmuls to chew on while it waits for exp results."""
    def new_qT(qb):
        qT = qp.tile([P, 4, 512], F32R, tag="qT", name=f"qT{qb % 2}")
        return qT

    def qkv_chains(qb, qT):
        yield from _qkv_strip_chains(tc, nc, xTr, wqk_sb, wv_sb, bqk_sb,
                                     kT_sb, v_sb, xp, ps_m, qb, qT)

    def run_chains(gen):
        for chain in gen:
            chain()

    qT_cur = new_qT(0)
    run_chains(qkv_chains(0, qT_cur))
    ctx_prev = None
    for qb in range(QB):
        qT_next = new_qT(qb + 1) if qb + 1 < QB else None
        fillers = []
        if qT_next is not None:
            fillers.append(qkv_chains(qb + 1, qT_next))
        if ctx_prev is not None:
            fillers.append(_outproj_chains(tc, nc, out, wo_sb, ctx_prev,
                                           osb, ps_m, qb - 1))
        ctx_prev = _attn(tc, nc, mbig, kT_sb, v_sb, bv_sb, qT_cur,
                         cxp, ep, bp, ps_s, ps_m, qb,
                         fillers=_roundrobin(fillers))
        qT_cur = qT_next
    run_chains(_outproj_chains(tc, nc, out, wo_sb, ctx_prev, osb, ps_m,
                               QB - 1))


def _roundrobin(gens):
    gens = list(gens)
    while gens:
        g = gens.pop(0)
        try:
            yield next(g)
            gens.append(g)
        except StopIteration:
            pass


def _qkv_strip_chains(tc, nc, xTr, wqk_sb, wv_sb, bqk_sb, kT_sb, v_sb,
                      xp, ps_m, qb, qT):
    """Yield one callable per accumulation chain (8 matmuls + a drain op)."""
    s0 = qb * 512
    state = {}

    def load_x():
        xs = xp.tile([P, 8, 512], F32R)
        nc.sync.dma_start(xs[:], xTr[:, :, s0:s0 + 512])
        state["xs"] = xs

    yield load_x

    def qk_chain(cb):
        xs = state["xs"]
        pq = ps_m.tile([P, 512], F32, tag="m")
        for e in range(8):
            nc.tensor.matmul(
                pq[:],
                (wqk_sb[:, e, cb * P:(cb + 1) * P]),
                (xs[:, e, :]),
                start=(e == 0), stop=(e == 7),
            )
        if cb < 4:
            dest = qT[:, cb, :]
        else:
            dest = kT_sb[:, cb - 4, s0:s0 + 512]
        if bqk_sb is not None:
            nc.vector.tensor_scalar_add(dest, pq[:],
                                        bqk_sb[:, cb:cb + 1])
        else:
            nc.vector.tensor_copy(dest, pq[:])

    def v_chain(sv):
        xs = state["xs"]
        j = s0 // P + sv
        pv = ps_m.tile([P, 512], F32, tag="m")
        for e in range(8):
            nc.tensor.matmul(
                pv[:],
                (xs[:, e, sv * P:(sv + 1) * P]),
                (wv_sb[:, e, :]),
                start=(e == 0), stop=(e == 7),
            )
        nc.vector.tensor_copy(
            v_sb[:, j, :, 0:HD],
            pv[:].rearrange("p (h d) -> p h d", h=GH),
        )

    # k and v chains first: the next qb's attention needs kT/v before qT
    for cb in (4, 5, 6, 7):
        yield (lambda cb=cb: qk_chain(cb))
    for sv in range(4):
        yield (lambda sv=sv: v_chain(sv))
    for cb in (0, 1, 2, 3):
        yield (lambda cb=cb: qk_chain(cb))


def _emit_pv(nc, pv2, v_sb, c, j, w0, ex, nj):
    for hp in range(2):
        nc.tensor.matmul(
            pv2[hp][0:HD + 1, w0:512],
            (v_sb[:, j, 2 * c + hp, :]),
            (ex[:, hp, w0:512]),
            start=(j == 0), stop=(j == nj - 1),
        )


def _outproj_chains(tc, nc, out, wo_sb, ctxT, osb, ps_m, qb):
    q0 = qb * 512
    for sb_i in range(4):
        for ob in range(2):
            def chain(sb_i=sb_i, ob=ob):
                po = ps_m.tile([P, 512], F32, tag="m")
                for cc in range(4):
                    nc.tensor.matmul(
                        po[:],
                        (ctxT[:, cc, sb_i * P:(sb_i + 1) * P]),
                        (wo_sb[:, cc, ob * 512:(ob + 1) * 512]),
                        start=(cc == 0), stop=(cc == 3),
                    )
                ot = osb.tile([P, 512], F32)
                nc.vector.tensor_copy(ot[:], po[:])
                nc.sync.dma_start(
                    out[q0 + sb_i * P:q0 + (sb_i + 1) * P,
                        ob * 512:(ob + 1) * 512],
                    ot[:],
                )
            yield chain


def _attn(tc, nc, mbig, kT_sb, v_sb, bv_sb, qT, cxp, ep, bp, ps_s, ps_m, qb, fillers=None):
    # ---- attention for this q-window ----
    # Heads 2c (SBUF partitions 0-63) and 2c+1 (64-127) are processed
    # together: their score matmuls land on PE row-groups (0,0)/(64,0)
    # and overlap in the array.
    ctxT = cxp.tile([P, 4, 512], F32R)       # [p, c, q]; ch = c*128+p
    nj = 4 * (qb + 1)
    fillers = iter(fillers) if fillers is not None else iter(())
    done = False
    n_iters = 4 * nj
    acc = 0.0
    per_iter = 18.0 / n_iters   # ~21 filler chains spread over the window

    def emit_fillers(force_all=False):
        nonlocal acc, done
        if done:
            return
        acc += per_iter
        while (acc >= 1.0 or force_all) and not done:
            acc -= 1.0
            try:
                next(fillers)()
            except StopIteration:
                done = True

    if True:
        for c in range(4):
            pv2 = [ps_m.tile([P, 512], F32, tag="m", name=f"pv{hp}")
                   for hp in range(2)]
            pend = None   # software-pipeline: PV trails scores by one j
            for j in range(nj):
                t = j - 4 * qb
                # Diagonal blocks only need q-columns >= t*128 (causality);
                # fp32r matmuls need N >= 256 for full rate, so the window
                # is capped at 256 wide minimum.
                w0 = 0 if t < 0 else min(t * P, 256)
                sp = ps_s.tile([P, 2, 512], F32)
                for hp in range(2):
                    p0 = 64 * hp
                    nc.tensor.matmul(
                        sp[:, hp, w0:512],
                        (kT_sb[p0:p0 + 64, c, j * P:(j + 1) * P]),
                        (qT[p0:p0 + 64, c, w0:512]),
                        start=True, stop=True,
                    )
                ex = ep.tile([P, 2, 512], F32R)
                nc.scalar.activation(ex[:, :, w0:512], sp[:, :, w0:512], AF.Exp)
                if t >= 0:
                    # mask multiply over the diagonal band (plus, for t=3,
                    # the fully-masked in-window left columns); columns right
                    # of the band are untouched (mask would be all-ones)
                    m0 = t * P if t < 3 else 256
                    m1 = (t + 1) * P if t < 3 else 512
                    nc.vector.tensor_mul(
                        ex[:, :, m0:m1],
                        ex[:, :, m0:m1],
                        mbig[:, None, m0 - 128 * t + 384:m1 - 128 * t + 384]
                        .to_broadcast((P, 2, m1 - m0)),
                    )
                if pend is not None:
                    _emit_pv(nc, pv2, v_sb, c, *pend, nj)
                pend = (j, w0, ex)
                emit_fillers()
            if pend is not None:
                _emit_pv(nc, pv2, v_sb, c, *pend, nj)
            # normalize: ctxT = pv[0:64] / pv[64] (+ v bias)
            for hp in range(2):
                p0 = 64 * hp
                pv_ps = pv2[hp]
                bc = bp.tile([64, 512], F32)
                nc.vector.reciprocal(bc[0:1, :], pv_ps[HD:HD + 1, :])
                nc.gpsimd.partition_broadcast(bc[:], bc[0:1, :])
                nc.vector.tensor_mul(ctxT[p0:p0 + 64, c, :], pv_ps[0:HD, :], bc[:])
                if bv_sb is not None:
                    nc.vector.tensor_scalar_add(
                        ctxT[p0:p0 + 64, c, :],
                        ctxT[p0:p0 + 64, c, :],
                        bv_sb[p0:p0 + 64, c:c + 1],
                    )
    emit_fillers(force_all=True)
    return ctxT


def _build_program(bench_iters=0, has_bias=True):
    nc = bacc.Bacc("TRN2", target_bir_lowering=False, debug=False,
                   num_devices=NCORES)
    xT = nc.dram_tensor("xT", [E, S], F32R, kind="ExternalInput").ap()
    wqkT = nc.dram_tensor("wqkT", [E, 2 * GC], F32R, kind="ExternalInput").ap()
    wvT = nc.dram_tensor("wvT", [E, GC], F32R, kind="ExternalInput").ap()
    woT = nc.dram_tensor("woT", [GC, E], F32R, kind="ExternalInput").ap()
    bqk = nc.dram_tensor("bqk", [8, P], F32, kind="ExternalInput").ap()
    bv = nc.dram_tensor("bv", [4, P], F32, kind="ExternalInput").ap()
    out = nc.dram_tensor("o", [S, E], F32, kind="ExternalOutput").ap()
    with tile.TileContext(nc) as tc:
        _emit(tc, nc, xT, wqkT, wvT, woT, bqk, bv, out, bench_iters=bench_iters,
              has_bias=has_bias)
    nc.compile()
    return nc


def _get_program(has_bias=True):
    if has_bias not in _program:
        _program[has_bias] = _build_program(has_bias=has_bias)
    return _program[has_bias]


def _make_in_maps(x, in_proj_w, in_proj_b, out_proj_w):
    scale = np.float32(1.0 / np.sqrt(HD))
    in_maps = []
    for c in range(NCORES):
        b, g = divmod(c, 2)
        lo, hi = g * GC, (g + 1) * GC
        wq = in_proj_w[lo:hi, :]
        wk = in_proj_w[E + lo:E + hi, :]
        wv = in_proj_w[2 * E + lo:2 * E + hi, :]
        wqkT = np.concatenate([wq.T * scale, wk.T], axis=1)
        wvT = np.ascontiguousarray(wv.T)
        woT = np.ascontiguousarray(out_proj_w[:, lo:hi].T)
        bqk = np.concatenate([in_proj_b[lo:hi] * scale,
                              in_proj_b[E + lo:E + hi]]).reshape(8, P)
        bvv = in_proj_b[2 * E + lo:2 * E + hi].reshape(4, P)
        xT = np.ascontiguousarray(x[b].T)
        in_maps.append({
            "xT": np.ascontiguousarray(xT, dtype=np.float32),
            "wqkT": np.ascontiguousarray(wqkT, dtype=np.float32),
            "wvT": wvT.astype(np.float32, copy=False),
            "woT": woT.astype(np.float32, copy=False),
            "bqk": np.ascontiguousarray(bqk, dtype=np.float32),
            "bv": np.ascontiguousarray(bvv, dtype=np.float32),
        })
    return in_maps


def _combine(results, out_proj_b):
    out = np.empty((B, S, E), dtype=np.float32)
    for b in range(B):
        out[b] = results[2 * b]["o"] + results[2 * b + 1]["o"]
    out += np.asarray(out_proj_b, dtype=np.float32)[None, None, :]
    return out


def kernel(x, in_proj_w, in_proj_b, out_proj_w, out_proj_b, _trace=False):
    x = np.asarray(x, dtype=np.float32)
    in_proj_w = np.asarray(in_proj_w, dtype=np.float32)
    in_proj_b = np.asarray(in_proj_b, dtype=np.float32)
    out_proj_w = np.asarray(out_proj_w, dtype=np.float32)
    out_proj_b = np.asarray(out_proj_b, dtype=np.float32)
    assert x.shape == (B, S, E), x.shape

    has_bias = bool(np.any(in_proj_b))
    nc = _get_program(has_bias=has_bias)
    in_maps = _make_in_maps(x, in_proj_w, in_proj_b, out_proj_w)
    res = run_bass_kernel_spmd(nc, in_maps, core_ids=list(range(NCORES)),
                               trace=_trace)
    out = _combine(res.results, out_proj_b)
    if _trace:
        return out, res
    return out

